# revision 36
# baseline (speedup 1.0000x reference)
"""ChebNet (K=4, 2 ChebConv layers + relu + log_softmax) on 8 trn2 NeuronCores.

Strategy (graph/data parallel, dense-ified SpMM on the TensorEngine):
  - The scaled-Laplacian propagation  prop(h) = A @ h  with
    A = -D^-1/2 Adj D^-1/2  is factored as  A = -diag(dis) @ Cnt @ diag(dis)
    where Cnt is the (dense-ified) edge-count matrix whose entries are small
    integers -- exactly representable in fp8e4m3.
  - Nodes are sharded 8 ways by destination. Each core keeps its Cnt^T shard
    [src=N_pad, dst=DLOC] fully SBUF-resident in fp8 (~12.6 MB) and computes
    prop outputs for its 1/8 of the nodes as a chain of PE matmuls in fp8
    DoubleRow mode: stationary lhsT = fp8 tile-PAIRS of dis*T_{k-1} (the
    all-gathered "g"), moving rhs = fp8 Cnt^T column pairs, accumulated fp32
    in PSUM.  DoubleRow contracts two 128-src-node tiles per instruction.
  - Between propagation steps the per-core [DLOC, F] fp8 result shards are
    exchanged with an 8-core AllGather (node-major), so every core always has
    the full g for the next contraction.
  - Per-order weight matmuls (T_k @ W[k]) run in bf16 on the PE, the
    Chebyshev recursion / relu / bias / log_softmax run in fp32 on DVE/ACT.

Everything is computed in a feature-transposed layout [F, node] per core;
the per-step transposes stage fp32 T tiles through PSUM and fuse the
dis-scale + fp8 quantization into the PSUM->SBUF copy on the ACT engine.
"""

import sys

sys.path.insert(0, "/opt/trn_rl_repo")

import numpy as np
import ml_dtypes

import concourse.bacc as bacc
import concourse.mybir as mybir
import concourse.tile as tile
from concourse.bass_utils import run_bass_kernel_spmd
from concourse.masks import make_identity

F32 = mybir.dt.float32
BF16 = mybir.dt.bfloat16
F8E4 = mybir.dt.float8e4

NCORES = 8
P = 128

# full-size problem geometry
N = 10000
F_IN = 128
HID = 128
C_OUT = 16
K_ORD = 4


class Geom:
    """Problem geometry. tiles_per_core src-tiles of 128 nodes per core."""

    def __init__(self, n_nodes, tiles_per_core, f_in=F_IN, hid=HID, c_out=C_OUT,
                 k_ord=K_ORD):
        self.n = n_nodes
        self.tpc = tiles_per_core          # src tiles per core (DLOC/128)
        self.dloc = tiles_per_core * P     # nodes per core (padded)
        self.npad = self.dloc * NCORES     # padded node count
        self.nt = self.npad // P           # total src tiles
        self.f = f_in
        self.hid = hid
        self.c = c_out
        self.k = k_ord
        assert self.npad >= n_nodes
        assert f_in == P and hid == P
        # psum chunking of the dloc free dim (max 512 fp32 per bank).
        # The first chunk is small so its recursion+stage+AllGather trigger
        # happens early in the step -- the 3-AG serial chain then finishes
        # before the next propagation needs each piece.
        self.chunks = []
        off = 0
        while off < self.dloc:
            sz = 256 if (off == 0 and self.dloc > 512) else \
                min(512, self.dloc - off)
            self.chunks.append((off, sz))
            off += sz
        # per-chunk local tile ranges (for the chunked exchange)
        self.ctiles = [(off // P, (off + sz) // P) for off, sz in self.chunks]
        # every chunk range must hold an even tile count (DoubleRow pairs)
        assert all((t1 - t0) % 2 == 0 for t0, t1 in self.ctiles)
        # a-tile groups (DMA/residency granularity): src tiles per group
        self.ag = next(a for a in (40, 16, 8) if self.nt % a == 0)
        assert self.nt % self.ag == 0
        self.n_agrp = self.nt // self.ag
        # g-tile groups: tpc src tiles per group (one core's shard)
        self.n_ggrp = NCORES


FULL = Geom(N, 10)  # 1280 nodes/core, npad=10240, 80 src tiles

# PE-warming filler matmuls issued while the AllGather is in flight, so the
# HAM clock gate stays at 8/8 for the next propagation.
N_WARM_STEP = 6
N_WARM_FIRST = 28


def build_nc(g: Geom):
    nc = bacc.Bacc("TRN2", target_bir_lowering=False, debug=False,
                   num_devices=NCORES)

    # ---- kernel I/O ----------------------------------------------------
    # fp8 Cnt^T shard, split per psum-chunk: [n_agrp, 128, ag, chunk_sz]
    a_in = [nc.dram_tensor(f"a_in_c{ci}", [g.n_agrp, P, g.ag, sz], F8E4,
                           kind="ExternalInput")
            for ci, (off, sz) in enumerate(g.chunks)]
    # initial g = fp8(dis * x), tiled [128, nt, f]
    g0_in = nc.dram_tensor("g0_in", [P, g.nt, g.f], F8E4, kind="ExternalInput")
    # local x^T shard fp32 (T0 in transposed layout)
    xt_in = nc.dram_tensor("xt_in", [P, g.dloc], F32, kind="ExternalInput")
    # broadcast +dis rows for the local shard (feature-major, recursion)
    disp_in = nc.dram_tensor("disp_in", [P, g.dloc], F32, kind="ExternalInput")
    # node-major dis for the local shard: dispt[p, t] = dis[node t*128+p]
    dispt_in = nc.dram_tensor("dispt_in", [P, g.tpc], F32,
                              kind="ExternalInput")
    w1_in = nc.dram_tensor("w1_in", [P, g.k, g.hid], BF16,
                           kind="ExternalInput")
    w2_in = nc.dram_tensor("w2_in", [P, g.k, g.c], BF16, kind="ExternalInput")
    # bias columns: col 0 = b1 (hid rows), col 1 = b2 (c rows)
    bb_in = nc.dram_tensor("bb_in", [P, 2], F32, kind="ExternalInput")

    out_dram = nc.dram_tensor("out", [g.dloc, g.c], F32, kind="ExternalOutput")

    n_ag = 5  # allgathers: L1 T1, L1 T2, h, L2 T1, L2 T2

    with tile.TileContext(nc) as tc:
        with (
            tc.tile_pool(name="pers", bufs=1) as pers,
            tc.tile_pool(name="work", bufs=1) as work,
            tc.tile_pool(name="psum", bufs=1, space="PSUM") as psp,
            tc.tile_pool(name="dram", bufs=1, space="DRAM") as drp,
        ):
            # ---- persistent SBUF ---------------------------------------
            a_sb = [[pers.tile([P, g.ag, sz], F8E4, tag=f"a{ci}_{i}",
                                name=f"a{ci}_{i}")
                     for i in range(g.n_agrp)]
                    for ci, (off, sz) in enumerate(g.chunks)]
            gbufC = [[pers.tile([P, g.n_ggrp, t1 - t0, g.f], F8E4,
                                tag=f"g{b}_{ci}", name=f"g{b}_{ci}")
                      for ci, (t0, t1) in enumerate(g.ctiles)]
                     for b in range(2)]
            t_sb = [pers.tile([P, g.dloc], F32, tag=f"t{i}", name=f"t{i}")
                    for i in range(3)]
            disp = pers.tile([P, g.dloc], F32, name="disp")
            dispt = pers.tile([P, g.tpc], F32, name="dispt")
            acc = pers.tile([P, g.dloc], F32, name="acc")
            # bf16 mirror of the current T_k (W-term rhs + transpose input)
            tbf = pers.tile([P, g.dloc], BF16, name="tbf")
            # bf16 T0 of the current layer (x for layer 1, h for layer 2)
            tb0 = pers.tile([P, g.dloc], BF16, name="tb0")
            tstage = pers.tile([P, g.tpc, g.f], F8E4, name="tstage")
            w1_sb = pers.tile([P, g.k, g.hid], BF16, name="w1_sb")
            w2_sb = pers.tile([P, g.k, g.c], BF16, name="w2_sb")
            bb_sb = pers.tile([P, 2], F32, name="bb_sb")
            idf32 = pers.tile([P, P], F32, name="idf32")
            idbf = pers.tile([P, P], BF16, name="idbf")

            # ---- DRAM bounce buffers for the collectives ---------------
            # partition-major payload: row = SBUF partition (node-in-tile),
            # col = (tile, feature). Makes both the stage-out DMA and the
            # post-AG g-load fully contiguous per partition.
            # the first exchange runs entirely behind the collective path's
            # ~80us cold-start wall, so nothing overlaps it: do it as ONE
            # AllGather instead of three serialized cold ones.
            ag_src_full = drp.tile([P, g.tpc * g.f], F8E4, name="ag_src_full")
            ag_dst_full = drp.tile([NCORES * P, g.tpc * g.f], F8E4,
                                   addr_space="Shared", name="ag_dst_full")
            ag_srcC = [[drp.tile([P, (t1 - t0) * g.f], F8E4,
                                 name=f"ag_src{i}_{ci}")
                        for ci, (t0, t1) in enumerate(g.ctiles)]
                       for i in range(n_ag)]
            ag_dstC = [[drp.tile([NCORES * P, (t1 - t0) * g.f], F8E4,
                                 addr_space="Shared", name=f"ag_dst{i}_{ci}")
                        for ci, (t0, t1) in enumerate(g.ctiles)]
                       for i in range(n_ag)]
            make_identity(nc, idf32[:])
            make_identity(nc, idbf[:])

            t_sb0 = pers.tile([P, g.dloc], F32, name="xt")

            # ---- loads, spread over five DGE queues, ordered by first
            # consumption: all g0 tiles first (every dst-chunk contraction
            # reads the full g), then Cnt chunk 0, small tensors, then the
            # remaining Cnt chunks.
            dges = [nc.sync, nc.scalar, nc.gpsimd]
            loads = []
            g0_4d = g0_in.ap().rearrange("p (j t) f -> p j t f", j=g.n_ggrp)
            for ci, (t0, t1) in enumerate(g.ctiles):
                loads.append((gbufC[0][ci], g0_4d[:, :, t0:t1, :]))
            for i in range(g.n_agrp):
                loads.append((a_sb[0][i], a_in[0][i]))
            loads.append((w1_sb, w1_in.ap()))
            loads.append((w2_sb, w2_in.ap()))
            loads.append((bb_sb, bb_in.ap()))
            loads.append((dispt, dispt_in.ap()))
            loads.append((disp, disp_in.ap()))
            loads.append((t_sb0, xt_in.ap()))
            for ci in range(1, len(g.chunks)):
                for i in range(g.n_agrp):
                    loads.append((a_sb[ci][i], a_in[ci][i]))
            for ld, (dst, src) in enumerate(loads):
                dges[ld % len(dges)].dma_start(dst[:], src)

            def warm(n_mm):
                """Keep the PE HAM clock gate open with dummy matmuls."""
                for _ in range(n_mm):
                    wp = psp.tile([P, 256], F32, space="PSUM", tag="warm",
                                  name="wp")
                    nc.tensor.matmul(wp[:, :P], lhsT=idbf[:], rhs=idbf[:],
                                     start=True, stop=True,
                                     skip_group_check=True)

            ag_idx = 0
            cur = 0  # g-buffer ping-pong index; gbuf[0] holds g(x)

            def chunk_tiles(off, sz):
                return range(off // P, (off + sz) // P)

            def stage_chunk(idx, ci, src_f32, off, sz, do_dma=True):
                """transpose fp32 tiles of src_f32 on the PE, fuse dis-scale
                + fp8 cast in the PSUM->SBUF copy, stage to ag_src."""
                for t in chunk_tiles(off, sz):
                    tpb = psp.tile([P, P], F32, space="PSUM", tag="tpb",
                                   name="tpb", bufs=2)
                    nc.tensor.transpose(out=tpb[:],
                                        in_=src_f32[:, t * P:(t + 1) * P],
                                        identity=idf32[:])
                    nc.scalar.mul(tstage[:, t, :], tpb[:], dispt[:, t:t + 1])
                if do_dma:
                    t0, t1 = off // P, (off + sz) // P
                    nc.scalar.dma_start(ag_srcC[idx][ci][:],
                                        tstage[:, t0:t1, :])

            # first exchange only: chunks c1+c2 go out as ONE AllGather
            # (they serialize cold behind the collective-path wall anyway)
            if len(g.chunks) > 1:
                bc_t0 = g.ctiles[1][0]
                bc_cols = (g.tpc - bc_t0) * g.f
                ag_src_bc = drp.tile([P, bc_cols], F8E4, name="ag_src_bc")
                ag_dst_bc = drp.tile([NCORES * P, bc_cols], F8E4,
                                     addr_space="Shared", name="ag_dst_bc")

            def allgather_bc(b_next):
                nc.scalar.dma_start(ag_src_bc[:], tstage[:, bc_t0:, :])
                nc.gpsimd.collective_compute(
                    "AllGather",
                    mybir.AluOpType.bypass,
                    replica_groups=[list(range(NCORES))],
                    ins=[ag_src_bc[:]],
                    outs=[ag_dst_bc[:]],
                )
                warm(N_WARM_FIRST)
                bc4d = ag_dst_bc[:, :].rearrange(
                    "(j p) (t f) -> p j t f", p=P, f=g.f)
                for ci in range(1, len(g.chunks)):
                    c0, c1 = g.ctiles[ci]
                    nc.sync.dma_start(
                        gbufC[b_next][ci][:],
                        bc4d[:, :, c0 - bc_t0:c1 - bc_t0, :])

            def allgather_full(b_next):
                """single AllGather of the whole staged shard (first
                exchange only: it sits behind the cold wall anyway)."""
                nc.scalar.dma_start(ag_src_full[:], tstage[:])
                nc.gpsimd.collective_compute(
                    "AllGather",
                    mybir.AluOpType.bypass,
                    replica_groups=[list(range(NCORES))],
                    ins=[ag_src_full[:]],
                    outs=[ag_dst_full[:]],
                )
                warm(N_WARM_FIRST)
                full4d = ag_dst_full[:, :].rearrange(
                    "(j p) (t f) -> p j t f", p=P, f=g.f)
                for ci, (t0, t1) in enumerate(g.ctiles):
                    nc.sync.dma_start(gbufC[b_next][ci][:],
                                      full4d[:, :, t0:t1, :])

            def allgather_chunk(idx, ci, b_next, last, first):
                nc.gpsimd.collective_compute(
                    "AllGather",
                    mybir.AluOpType.bypass,
                    replica_groups=[list(range(NCORES))],
                    ins=[ag_srcC[idx][ci][:]],
                    outs=[ag_dstC[idx][ci][:]],
                )
                if last:
                    warm(N_WARM_FIRST if idx == 0 else N_WARM_STEP)
                # one batched g-load on sync: a DMA on the gpsimd queue would
                # delay the next collective trigger behind this AG's
                # completion, and one on the scalar queue would block the ACT
                # stage chain.
                nc.sync.dma_start(
                    gbufC[b_next][ci][:],
                    ag_dstC[idx][ci][:, :]
                    .rearrange("(j p) (t f) -> p j t f", p=P, f=g.f),
                )

            def w_term_chunk(w_sb, k, rhs_bf, cdim, off, sz):
                """acc[0:cdim, chunk] (+)= (T_k @ W[k])^T, bf16 matmul."""
                wt = psp.tile([P, 512], F32, space="PSUM", tag="wt",
                              name="wt", bufs=2)
                nc.tensor.matmul(
                    wt[:cdim, :sz],
                    lhsT=w_sb[:, k, :],
                    rhs=rhs_bf[:, off:off + sz],
                    start=True, stop=True,
                )
                if k == 0:
                    nc.vector.tensor_copy(acc[:cdim, off:off + sz],
                                          wt[:cdim, :sz])
                else:
                    nc.vector.tensor_add(acc[:cdim, off:off + sz],
                                         acc[:cdim, off:off + sz],
                                         wt[:cdim, :sz])

            z_all = work.tile([P, g.tpc, g.c], F32, name="z_all")
            m_all = work.tile([P, g.tpc, 1], F32, name="m_all")
            e_all = work.tile([P, g.tpc, g.c], F32, name="e_all")
            s_all = work.tile([P, g.tpc, 1], F32, name="s_all")
            o_all = work.tile([P, g.tpc, g.c], F32, name="o_all")
            out_ap = out_dram.ap().rearrange("(t p) c -> p t c", p=P)

            def tail_softmax(ci, off, sz):
                """log_softmax + output DMA for one finished chunk."""
                t0, t1 = off // P, (off + sz) // P
                z = z_all[:, t0:t1, :]
                m = m_all[:, t0:t1, :]
                e = e_all[:, t0:t1, :]
                s = s_all[:, t0:t1, :]
                o = o_all[:, t0:t1, :]
                nt = t1 - t0
                nc.vector.tensor_reduce(out=m[:, :, 0], in_=z,
                                        axis=mybir.AxisListType.X,
                                        op=mybir.AluOpType.max)
                nc.vector.tensor_tensor(out=e, in0=z,
                                        in1=m.to_broadcast([P, nt, g.c]),
                                        op=mybir.AluOpType.subtract)
                nc.scalar.activation(e, e, mybir.ActivationFunctionType.Exp)
                nc.vector.tensor_reduce(out=s[:, :, 0], in_=e,
                                        axis=mybir.AxisListType.X,
                                        op=mybir.AluOpType.add)
                nc.scalar.activation(s, s, mybir.ActivationFunctionType.Ln)
                nc.vector.tensor_add(s, s, m)
                nc.vector.tensor_tensor(out=o, in0=z,
                                        in1=s.to_broadcast([P, nt, g.c]),
                                        op=mybir.AluOpType.subtract)
                nc.sync.dma_start(out_ap[:, t0:t1, :], o)

            # ---- the two ChebConv layers -------------------------------
            for layer in range(2):
                w_sb = w1_sb if layer == 0 else w2_sb
                cdim = g.hid if layer == 0 else g.c
                if layer == 1:
                    # T0 term for layer 2 (tb0 = bf16 h): fills the h-AG wait
                    for (off, sz) in g.chunks:
                        w_term_chunk(w_sb, 0, tb0, cdim, off, sz)
                # contraction order: chunk-0 tile pairs (earliest AllGather)
                # first, then chunk 1, then chunk 2.
                pair_order = [(gci, j, t0 + 2 * p)
                              for gci, (t0, t1) in enumerate(g.ctiles)
                              for j in range(g.n_ggrp)
                              for p in range((t1 - t0) // 2)]
                n_pairs = len(pair_order)
                for k in range(1, g.k):
                    tk = t_sb[k % 3]
                    tk2 = (t_sb[(k - 2) % 3] if k >= 3 else
                           (t_sb0 if layer == 0 else t_sb[0])) \
                        if k >= 2 else None
                    do_stage = k < g.k - 1  # T3 needs no exchange
                    do_ag = do_stage or layer == 0

                    def tail_chunk(ci, off, sz, k=k, layer=layer, tk=tk):
                        """staging (critical: feeds the AllGather) first,
                        then the bf16 cast + W-term off the critical path."""
                        if do_stage:
                            merged = ag_idx == 0 and len(g.chunks) > 1
                            stage_chunk(ag_idx, ci, tk, off, sz,
                                        do_dma=(not merged or ci == 0))
                            nc.scalar.copy(tbf[:, off:off + sz],
                                           tk[:, off:off + sz])
                            w_term_chunk(w_sb, k, tbf, cdim, off, sz)
                            if not merged:
                                allgather_chunk(
                                    ag_idx, ci, 1 - cur,
                                    last=(ci == len(g.chunks) - 1),
                                    first=(ag_idx == 0))
                            elif ci == 0:
                                allgather_chunk(ag_idx, 0, 1 - cur,
                                                last=False, first=True)
                            elif ci == len(g.chunks) - 1:
                                allgather_bc(1 - cur)
                            return
                        nc.scalar.copy(tbf[:, off:off + sz],
                                       tk[:, off:off + sz])
                        w_term_chunk(w_sb, k, tbf, cdim, off, sz)
                        if layer == 0:
                            # layer end: h = relu(acc + b1), fp32 + bf16
                            nc.scalar.activation(
                                t_sb[0][:, off:off + sz],
                                acc[:, off:off + sz],
                                mybir.ActivationFunctionType.Relu,
                                bias=bb_sb[:, 0:1], scale=1.0)
                            stage_chunk(ag_idx, ci, t_sb[0], off, sz)
                            nc.scalar.activation(
                                tb0[:, off:off + sz],
                                acc[:, off:off + sz],
                                mybir.ActivationFunctionType.Relu,
                                bias=bb_sb[:, 0:1], scale=1.0)
                        else:
                            # final layer: bias2 + transpose to node-major
                            nc.scalar.activation(
                                acc[:g.c, off:off + sz],
                                acc[:g.c, off:off + sz],
                                mybir.ActivationFunctionType.Identity,
                                bias=bb_sb[:g.c, 1:2], scale=1.0)
                            for t in chunk_tiles(off, sz):
                                zp = psp.tile([P, g.c], F32, space="PSUM",
                                              tag="tpb", name="zp", bufs=2)
                                nc.tensor.transpose(
                                    out=zp[:],
                                    in_=acc[:g.c, t * P:(t + 1) * P],
                                    identity=idf32[:g.c, :g.c])
                                nc.vector.tensor_copy(z_all[:, t, :], zp[:])
                            tail_softmax(ci, off, sz)
                        if do_ag:
                            allgather_chunk(ag_idx, ci, 1 - cur,
                                            last=(ci == len(g.chunks) - 1),
                                            first=(ag_idx == 0))

                    for ci, (off, sz) in enumerate(g.chunks):
                        pp = psp.tile([P, 512], F32, space="PSUM", tag="pp",
                                      name="pp", bufs=2)
                        for n_i, (gci, j, t) in enumerate(pair_order):
                            gi = j * g.tpc + t
                            ts0 = g.ctiles[gci][0]
                            lhs = gbufC[cur][gci][:, j,
                                                  t - ts0:t - ts0 + 2, :]
                            rhs = a_sb[ci][gi // g.ag][:,
                                                       gi % g.ag:gi % g.ag + 2,
                                                       :]
                            nc.tensor.matmul(
                                pp[:, :sz],
                                lhsT=lhs,
                                rhs=rhs,
                                start=(n_i == 0),
                                stop=(n_i == n_pairs - 1),
                                perf_mode=mybir.MatmulPerfMode.DoubleRow,
                            )
                        if layer == 0 and k == 1 and ci == 0:
                            # layer-1 T0 W-term: off the critical path, PE
                            # is load-bound here anyway
                            nc.scalar.copy(tb0[:], t_sb0[:])
                            for (off0, sz0) in g.chunks:
                                w_term_chunk(w_sb, 0, tb0, cdim, off0, sz0)
                        # Chebyshev recursion (fp32, on DVE)
                        if k == 1:
                            nc.vector.scalar_tensor_tensor(
                                out=tk[:, off:off + sz],
                                in0=pp[:, :sz],
                                scalar=-1.0,
                                in1=disp[:, off:off + sz],
                                op0=mybir.AluOpType.mult,
                                op1=mybir.AluOpType.mult)
                        else:
                            nc.vector.scalar_tensor_tensor(
                                out=tk[:, off:off + sz],
                                in0=pp[:, :sz],
                                scalar=-2.0,
                                in1=disp[:, off:off + sz],
                                op0=mybir.AluOpType.mult,
                                op1=mybir.AluOpType.mult)
                            nc.vector.tensor_sub(
                                tk[:, off:off + sz],
                                tk[:, off:off + sz],
                                tk2[:, off:off + sz])
                        tail_chunk(ci, off, sz)
                    if do_ag:
                        ag_idx += 1
                        cur = 1 - cur

    nc.compile()
    return nc


def host_prep(g: Geom, x, edge_index, W1, b1, W2, b2):
    """Build the per-core input maps (sharding + dense-ification)."""
    n = g.n
    src = np.asarray(edge_index[0], dtype=np.int64)
    dst = np.asarray(edge_index[1], dtype=np.int64)
    deg = np.bincount(src, minlength=n).astype(np.float64)
    dis = np.where(deg > 0, 1.0 / np.sqrt(np.maximum(deg, 1e-12)), 0.0)

    # dense-ified edge-count matrix, transposed: cnt_t[s, d]
    cnt_t = np.zeros((g.npad, g.npad), dtype=np.float32)
    np.add.at(cnt_t, (src, dst), 1.0)

    dis_pad = np.zeros(g.npad, dtype=np.float32)
    dis_pad[:n] = dis.astype(np.float32)
    x_pad = np.zeros((g.npad, g.f), dtype=np.float32)
    x_pad[:n] = np.asarray(x, dtype=np.float32)

    g0 = dis_pad[:, None] * x_pad  # [npad, f]
    g0_tiles = (g0.reshape(g.nt, P, g.f).transpose(1, 0, 2)
                .astype(ml_dtypes.float8_e4m3))  # [128, nt, f]

    w1 = np.ascontiguousarray(
        np.asarray(W1, np.float32).transpose(1, 0, 2)
    ).astype(ml_dtypes.bfloat16)  # [P, k, hid]
    w2 = np.ascontiguousarray(
        np.asarray(W2, np.float32).transpose(1, 0, 2)
    ).astype(ml_dtypes.bfloat16)  # [P, k, c]
    bb = np.zeros((P, 2), np.float32)
    bb[:g.hid, 0] = np.asarray(b1, np.float32)
    bb[:g.c, 1] = np.asarray(b2, np.float32)

    in_maps = []
    for c in range(NCORES):
        lo, hi = c * g.dloc, (c + 1) * g.dloc
        a_c = (cnt_t[:, lo:hi].astype(ml_dtypes.float8_e4m3)
               .reshape(g.n_agrp, g.ag, P, g.dloc).transpose(0, 2, 1, 3))
        a_chunks = [np.ascontiguousarray(a_c[:, :, :, off:off + sz])
                    for (off, sz) in g.chunks]
        xt = np.ascontiguousarray(x_pad[lo:hi].T)          # [128, dloc]
        d_loc = dis_pad[lo:hi]
        disp = np.ascontiguousarray(
            np.broadcast_to(d_loc[None, :], (P, g.dloc))).astype(np.float32)
        dispt = np.ascontiguousarray(
            d_loc.reshape(g.tpc, P).T).astype(np.float32)  # [128, tpc]
        im = {f"a_in_c{ci}": a_chunks[ci] for ci in range(len(g.chunks))}
        im.update({
            "g0_in": np.ascontiguousarray(g0_tiles),
            "xt_in": xt,
            "disp_in": disp,
            "dispt_in": dispt,
            "w1_in": w1,
            "w2_in": w2,
            "bb_in": bb,
        })
        in_maps.append(im)
    return in_maps


_CACHED_NC = None


def _get_nc():
    global _CACHED_NC
    if _CACHED_NC is None:
        _CACHED_NC = build_nc(FULL)
    return _CACHED_NC


def _enable_ldw_opt():
    """The default axon compile flags pass --enable-ldw-opt=false, which
    serializes every LDWEIGHTS with its MATMUL (~+107ns per matmul). Our
    kernel is a long stream of ldweights+matmul pairs, so re-enable it."""
    try:
        from concourse.compiler_utils import (get_compiler_flags,
                                              set_compiler_flags)
        flags = get_compiler_flags()
        new = [f.replace("--enable-ldw-opt=false", "--enable-ldw-opt=true")
               for f in flags]
        if new != flags:
            set_compiler_flags(new)
    except Exception:
        pass


def kernel(x, edge_index, W1, b1, W2, b2, _profile=False):
    g = FULL
    _enable_ldw_opt()
    in_maps = host_prep(g, x, edge_index, W1, b1, W2, b2)
    nc = _get_nc()
    res = run_bass_kernel_spmd(nc, in_maps, list(range(NCORES)),
                               trace=_profile)
    out = np.concatenate([res.results[c]["out"] for c in range(NCORES)], 0)
    out = out[:g.n].astype(np.float32)
    if _profile:
        kernel.last_result = res
    return out
